# revision 7
# baseline (speedup 1.0000x reference)
"""Trainium2 Bass kernel for nn_AsymetricKernel (linear attention w/ InstanceNorm + 2D rotary).

Sharding: 8 cores = 4 batches x 2 head-groups (4 heads each). Fully independent
per core -- no collectives.

Per-core dataflow (PE compute in bf16, fp32 PSUM accumulation):
  - q is projected ONCE per head-pair in transposed layout [d, n]. Rotary's
    rotate-half is moved out of the data path entirely: u = (q*cos)^T D +
    (q*sin_sigma)^T D' where sin_sigma is a host-permuted sin table and
    D' = P^T D is the row-swapped dots matrix (one tiny PE matmul).
  - k/v are projected in token-partition layout [tok, k|v] with per-head
    (negated) mean columns from a parallel small matmul. Variance comes from
    one ACT square pass (PSUM->SBUF) + one grouped DVE reduce; the stats soup
    is fused with scalar_tensor_tensor ops, all unit-stride.
  - InstanceNorm scales combine as rc = 1/(sd_k*sd_v) applied to k only; v's
    mean rides as a -mv column in a stride-66 v layout so one matmul per
    (chunk, head-pair) accumulates both dots and the mean correction, which is
    folded back per-partition in the finalize.
  - k_rot = (k*rc + mcs)*cos + sigma-swap(...)*sin is pre-added so dots needs
    a single accumulating matmul per (chunk, head-pair).
  - Pass 2 streams 512-token chunks of t1/t2 against the stationary 128x128
    dots matrices: u^T accumulates in PSUM [e, tok], copied to bf16 and
    DMA'd out transposed; the host transposes back and upcasts.
"""

import numpy as np
import ml_dtypes

B, N, DIM, H, DH = 4, 8192, 512, 8, 64
HG = 2              # head groups (cores per batch) / head-pairs per core
HPG = H // HG       # heads per group = 4
E = HPG * DH        # 256 output cols per core
NT = 16             # n-tiles of 512
CPT = 4             # 128-chunks per n-tile
NCHUNK = NT * CPT   # 64
CC = DIM // 128     # 4 contraction chunks
VW = DH + 2         # v columns per head in dots rhs: 64 vals + (-mv) + pad

_cache = {}


def _build_program():
    import concourse.tile as tile
    from concourse import bacc, mybir
    from contextlib import ExitStack

    f32 = mybir.dt.float32
    bf16 = mybir.dt.bfloat16
    AX = mybir.AxisListType
    OP = mybir.AluOpType

    nc = bacc.Bacc(target_bir_lowering=False)
    uxT = nc.declare_dram_parameter("uxT", [NT, 128, CC * 512], bf16, isOutput=False)
    wq = nc.declare_dram_parameter("wq", [128, CC * E], bf16, isOutput=False)
    wkv = nc.declare_dram_parameter("wkv", [128, CC * 2 * E], bf16, isOutput=False)
    wm = nc.declare_dram_parameter("wm", [128, CC * 2 * HPG], bf16, isOutput=False)
    ctab = nc.declare_dram_parameter("ctab", [128, 2, N], bf16, isOutput=False)
    cptab = nc.declare_dram_parameter("cptab", [128, 2, NCHUNK * DH], bf16, isOutput=False)
    Pm = nc.declare_dram_parameter("Pm", [128, 128], bf16, isOutput=False)
    out = nc.declare_dram_parameter("out", [HG * 128, N], bf16, isOutput=True)

    with ExitStack() as ctx:
        tc = ctx.enter_context(tile.TileContext(nc))
        consts = ctx.enter_context(tc.tile_pool(name="consts", bufs=1))
        store = ctx.enter_context(tc.tile_pool(name="store", bufs=1))

        # ---- persistent SBUF ----
        wq_sb = consts.tile([128, CC, E], bf16)
        nc.sync.dma_start(wq_sb[:].rearrange("p c e -> p (c e)"), wq[:])
        wkv_sb = consts.tile([128, CC, 2 * E], bf16)
        nc.sync.dma_start(wkv_sb[:].rearrange("p c e -> p (c e)"), wkv[:])
        wm_sb = consts.tile([128, CC, 2 * HPG], bf16)
        nc.sync.dma_start(wm_sb[:].rearrange("p c e -> p (c e)"), wm[:])
        P_sb = consts.tile([128, 128], bf16)
        nc.sync.dma_start(P_sb[:], Pm[:])

        t1T_sb = store.tile([128, HG, N], bf16)   # (q * cos)^T per head-pair
        t2T_sb = store.tile([128, HG, N], bf16)   # (q * sin_sigma)^T
        D_sb = store.tile([128, HG, 128], bf16)   # block-diag dots per pair
        Dp_sb = store.tile([128, HG, 128], bf16)  # row-swapped dots
        gA = store.tile([128, HG], f32)           # -mv correction cols
        nc.vector.memset(D_sb[:], 0.0)

        with ExitStack() as p1:
            uxp = p1.enter_context(tc.tile_pool(name="uxp", bufs=3))
            ctp = p1.enter_context(tc.tile_pool(name="ctp", bufs=3))
            cpp = p1.enter_context(tc.tile_pool(name="cpp", bufs=3))
            sqp = p1.enter_context(tc.tile_pool(name="sqp", bufs=2))
            ksp = p1.enter_context(tc.tile_pool(name="ksp", bufs=2))
            kch = p1.enter_context(tc.tile_pool(name="kch", bufs=2))
            kdp = p1.enter_context(tc.tile_pool(name="kdp", bufs=4))
            stp = p1.enter_context(tc.tile_pool(name="stp", bufs=2))
            qps = p1.enter_context(tc.tile_pool(name="qps", bufs=2, space="PSUM"))
            kvps = p1.enter_context(tc.tile_pool(name="kvps", bufs=2, space="PSUM"))
            mps = p1.enter_context(tc.tile_pool(name="mps", bufs=1, space="PSUM"))
            dps = p1.enter_context(tc.tile_pool(name="dps", bufs=1, space="PSUM"))

            # dots accumulator: head-pair eb in partitions (g*64+d), columns
            # [g*66 + (v-dims 64, -mv col, pad)]; off-diagonal blocks are junk.
            dA = dps.tile([128, HG, 2 * VW], f32)

            pend = {}

            def emit_dots(t):
                krot_t, vD_t = pend.pop(t)
                for ci in range(CPT):
                    for eb in range(HG):
                        # start/stop once per BANK: start=True clears has_written
                        # bank-wide, so per-eb starts would wipe the other
                        # group's partial accumulation.
                        nc.tensor.matmul(
                            dA[:, eb, :], krot_t[:, ci, 2 * eb:2 * eb + 2, :],
                            vD_t[:, ci, 2 * eb:2 * eb + 2, :],
                            start=(t == 0 and ci == 0 and eb == 0),
                            stop=(t == NT - 1 and ci == CPT - 1 and eb == HG - 1))

            for nt in range(NT):
                ns = nt * 512
                ux_t = uxp.tile([128, CC, 512], bf16)
                nc.sync.dma_start(ux_t[:].rearrange("p c n -> p (c n)"), uxT[nt, :, :])
                ct_t = ctp.tile([128, 2, 512], bf16)
                nc.sync.dma_start(ct_t[:], ctab[:, :, ns:ns + 512])
                cp_t = cpp.tile([128, 2, CPT, DH], bf16)
                nc.sync.dma_start(cp_t[:].rearrange("p a c d -> p a (c d)"),
                                  cptab[:, :, nt * CPT * DH:(nt + 1) * CPT * DH])

                # ---- transposed q projection + rotary premuls ----
                for eb in range(HG):
                    qp = qps.tile([128, 512], f32, tag="qp")
                    for cc in range(CC):
                        nc.tensor.matmul(
                            qp[:], wq_sb[:, cc, eb * 128:(eb + 1) * 128],
                            ux_t[:, cc, :],
                            start=(cc == 0), stop=(cc == CC - 1))
                    nc.vector.tensor_mul(
                        t1T_sb[:, eb, ns:ns + 512], qp[:], ct_t[:, 0, :])
                    nc.vector.tensor_mul(
                        t2T_sb[:, eb, ns:ns + 512], qp[:], ct_t[:, 1, :])

                # ---- k/v + mean projections, PSUM->SBUF per half-tile ----
                mp = mps.tile([128, CPT, 2 * HPG], f32, tag="mp")
                sq = sqp.tile([128, CPT, 2 * HPG, DH], bf16, tag="sq")
                ksb = ksp.tile([128, CPT, HPG, DH], bf16, tag="ksb")
                vD = kdp.tile([128, CPT, HPG, VW], bf16, tag="vD")
                for hf in range(2):
                    kvp = kvps.tile([128, 2, 2 * E], f32, tag="kvp")
                    for ci2 in range(2):
                        ci = hf * 2 + ci2
                        for cc in range(CC):
                            nc.tensor.matmul(
                                kvp[:, ci2, :], ux_t[:, cc, ci * 128:(ci + 1) * 128],
                                wkv_sb[:, cc, :],
                                start=(cc == 0), stop=(cc == CC - 1))
                            nc.tensor.matmul(
                                mp[:, ci, :], ux_t[:, cc, ci * 128:(ci + 1) * 128],
                                wm_sb[:, cc, :],
                                start=(cc == 0), stop=(cc == CC - 1))
                    hsl = slice(2 * hf, 2 * hf + 2)
                    nc.scalar.square(sq[:, hsl],
                                     kvp[:].rearrange("p c (g d) -> p c g d", g=2 * HPG))
                    nc.scalar.copy(ksb[:, hsl],
                                   kvp[:, :, 0:E].rearrange("p c (g d) -> p c g d", g=HPG))
                    nc.scalar.copy(vD[:, hsl, :, 0:DH],
                                   kvp[:, :, E:2 * E].rearrange("p c (g d) -> p c g d", g=HPG))

                # ---- whole-tile stats, all unit-stride ----
                mpc = stp.tile([128, CPT, 2 * HPG], f32, tag="mpc")
                nc.scalar.copy(mpc[:], mp[:])
                ss = stp.tile([128, CPT, 2 * HPG], f32, tag="ss")
                nc.vector.tensor_reduce(out=ss[:], in_=sq[:], axis=AX.X, op=OP.add)
                negmu2 = stp.tile([128, CPT, 2 * HPG], f32, tag="nmu")
                nc.vector.scalar_tensor_tensor(
                    negmu2[:], mpc[:], -1.0, mpc[:], OP.mult, OP.mult)
                var = stp.tile([128, CPT, 2 * HPG], f32, tag="var")
                nc.vector.scalar_tensor_tensor(
                    var[:], ss[:], 1.0 / DH, negmu2[:], OP.mult, OP.add)
                sd = stp.tile([128, CPT, 2 * HPG], f32, tag="sd")
                nc.scalar.sqrt(sd[:], var[:])
                sdp = stp.tile([128, CPT, HPG], f32, tag="sdp")
                nc.vector.tensor_mul(sdp[:], sd[:, :, 0:HPG], sd[:, :, HPG:2 * HPG])
                rc = stp.tile([128, CPT, HPG], f32, tag="rc")
                nc.vector.reciprocal(rc[:], sdp[:])
                mcs = stp.tile([128, CPT, HPG], f32, tag="mcs")
                nc.gpsimd.tensor_mul(mcs[:], mpc[:, :, 0:HPG], rc[:])
                # v mean col (+ pad col, same value, junk output never read)
                nc.scalar.copy(vD[:, :, :, DH:DH + 2],
                               mpc[:, :, HPG:2 * HPG].unsqueeze(-1).broadcast_to(
                                   [128, CPT, HPG, 2]))

                # ---- k chain: normalize + rotary, pre-added ----
                ktn = kch.tile([128, CPT, HPG, DH], bf16, tag="ktn")
                nc.gpsimd.tensor_mul(
                    ktn[:], ksb[:],
                    rc[:].unsqueeze(-1).broadcast_to([128, CPT, HPG, DH]))
                nc.gpsimd.tensor_add(
                    ktn[:], ktn[:],
                    mcs[:].unsqueeze(-1).broadcast_to([128, CPT, HPG, DH]))
                k1 = kch.tile([128, CPT, HPG, DH], bf16, tag="k1")
                nc.vector.tensor_mul(
                    k1[:], ktn[:],
                    cp_t[:, 0].unsqueeze(2).broadcast_to([128, CPT, HPG, DH]))
                k2 = kch.tile([128, CPT, HPG, DH], bf16, tag="k2")
                ktn_r = ktn[:].rearrange("p c g (b h s) -> p c g b h s", b=2, h=2, s=16)
                k2_r = k2[:].rearrange("p c g (b h s) -> p c g b h s", b=2, h=2, s=16)
                spb = cp_t[:, 1].rearrange("p c (b h s) -> p c b h s", b=2, h=2, s=16
                                           ).unsqueeze(2).broadcast_to(
                                               [128, CPT, HPG, 2, 2, 16])
                # ISA TensorTensor allows max 3 free dims -> one op per (b, h)
                for bb in range(2):
                    for hh in range(2):
                        nc.vector.tensor_mul(
                            k2_r[:, :, :, bb, hh, :],
                            ktn_r[:, :, :, bb, 1 - hh, :],
                            spb[:, :, :, bb, hh, :])
                krot = kdp.tile([128, CPT, HPG, DH], bf16, tag="krot")
                nc.gpsimd.tensor_add(krot[:], k1[:], k2[:])
                pend[nt] = (krot, vD)
                # 2-tile lag: dots(nt) only needs krot(nt) ~2 tiles later, so
                # the PE never stalls on the elementwise chain (keeps HAM warm)
                if nt > 1:
                    emit_dots(nt - 2)
            emit_dots(NT - 2)
            emit_dots(NT - 1)

            # ---- finalize dots: fold -mv col, scale 1/N, cast bf16 ----
            for eb in range(HG):
                for g in range(2):
                    psl = slice(g * 64, (g + 1) * 64)
                    col = g * VW + DH
                    nc.scalar.copy(gA[psl, eb:eb + 1], dA[psl, eb, col:col + 1])
            for eb in range(HG):
                for g in range(2):
                    psl = slice(g * 64, (g + 1) * 64)
                    nc.vector.tensor_scalar(
                        out=D_sb[psl, eb, g * 64:(g + 1) * 64],
                        in0=dA[psl, eb, g * VW:g * VW + DH],
                        scalar1=gA[psl, eb:eb + 1], scalar2=1.0 / N,
                        op0=OP.add, op1=OP.mult)

        # ---- pass 2: D' = P^T D, then u^T = D^T t1 + D'^T t2 ----
        with ExitStack() as p2:
            dpps = p2.enter_context(tc.tile_pool(name="dpps", bufs=1, space="PSUM"))
            ups = p2.enter_context(tc.tile_pool(name="ups", bufs=3, space="PSUM"))
            uout = p2.enter_context(tc.tile_pool(name="uout", bufs=4))
            dps2 = dpps.tile([128, HG, 128], f32)
            for eb in range(HG):
                nc.tensor.matmul(dps2[:, eb, :], P_sb[:], D_sb[:, eb, :],
                                 start=True, stop=True)
                nc.vector.tensor_copy(Dp_sb[:, eb, :], dps2[:, eb, :])
            for eb in range(HG):
                for cs in range(NT):
                    nsl = slice(cs * 512, (cs + 1) * 512)
                    up = ups.tile([128, 512], f32, tag="up")
                    nc.tensor.matmul(up[:], D_sb[:, eb, :], t1T_sb[:, eb, nsl],
                                     start=True, stop=False)
                    nc.tensor.matmul(up[:], Dp_sb[:, eb, :], t2T_sb[:, eb, nsl],
                                     start=False, stop=True)
                    u_sb = uout.tile([128, 512], bf16)
                    if cs % 2 == 0:
                        nc.vector.tensor_copy(u_sb[:], up[:])
                    else:
                        nc.scalar.copy(u_sb[:], up[:])
                    nc.sync.dma_start(out[eb * 128:(eb + 1) * 128, nsl], u_sb[:])

    nc.finalize()
    return nc


def _host_prep(u_x, pos_x, Wq, Wk, Wv):
    bf = ml_dtypes.bfloat16
    invf = 1.0 / 10000.0 ** (np.arange(0, 32, 2, dtype=np.float64) / 32)
    t64 = pos_x[0].astype(np.float64) * 64.0
    fx = t64[:, 0:1] * invf[None, :]
    fy = t64[:, 1:2] * invf[None, :]
    cx, sx = np.cos(fx), np.sin(fx)
    cy, sy = np.cos(fy), np.sin(fy)
    cosf = np.concatenate([cx, cx, cy, cy], 1).astype(np.float32)     # c[n, 64]
    stil = np.concatenate([-sx, sx, -sy, sy], 1).astype(np.float32)   # rho*sin
    sig = np.arange(64) ^ 16

    def chunked(t):  # [N, 64] -> [128, NCHUNK*64], partition = token % 128
        return np.ascontiguousarray(
            t.reshape(NCHUNK, 128, DH).transpose(1, 0, 2).reshape(128, -1))

    cptab = np.stack([chunked(cosf), chunked(stil)], axis=1).astype(bf)
    ctab = np.stack([np.tile(cosf.T, (2, 1)),
                     np.tile(stil[:, sig].T, (2, 1))], axis=1).astype(bf)
    Pmh = np.eye(128, dtype=np.float32)[np.arange(128) ^ 16].astype(bf)

    def wlayout(wT):  # [512, E'] -> [128, CC*E'] partition-native
        Ep = wT.shape[1]
        return np.ascontiguousarray(
            wT.reshape(CC, 128, Ep).transpose(1, 0, 2).reshape(128, -1)).astype(bf)

    in_maps = []
    for b in range(B):
        uxtb = np.ascontiguousarray(
            u_x[b].reshape(NT, 512, CC, 128).transpose(0, 3, 2, 1).reshape(
                NT, 128, CC * 512)).astype(bf)
        for hg in range(HG):
            sl = slice(hg * E, (hg + 1) * E)
            wbar_k = Wk[sl].reshape(HPG, DH, DIM).mean(1).T            # [512, 4]
            wbar_v = Wv[sl].reshape(HPG, DH, DIM).mean(1).T
            in_maps.append({
                "uxT": uxtb,
                "wq": wlayout(Wq[sl].T.astype(np.float32)),
                "wkv": wlayout(np.concatenate([Wk[sl].T, Wv[sl].T], 1)),
                "wm": wlayout(np.concatenate([-wbar_k, -wbar_v], 1)),
                "ctab": ctab, "cptab": cptab, "Pm": Pmh,
            })
    return in_maps


def kernel(u_x, pos_x, Wq, Wk, Wv, _trace=False, _trace_dir=None):
    from concourse.bass_utils import run_bass_kernel_spmd

    if "nc" not in _cache:
        _cache["nc"] = _build_program()
    nc = _cache["nc"]

    in_maps = _host_prep(
        np.asarray(u_x, np.float32), np.asarray(pos_x, np.float32),
        np.asarray(Wq, np.float32), np.asarray(Wk, np.float32),
        np.asarray(Wv, np.float32))

    kw = {}
    if _trace:
        kw = {"trace": True, "tmpdir": _trace_dir}
    res = run_bass_kernel_spmd(nc, in_maps, core_ids=list(range(8)), **kw)
    _cache["last_result"] = res

    out = np.empty((B, N, H * DH), np.float32)
    for i in range(8):
        b, hg = divmod(i, HG)
        out[b, :, hg * E:(hg + 1) * E] = np.asarray(
            res.results[i]["out"], np.float32).T
    return out


# revision 12
# speedup vs baseline: 1.2087x; 1.2087x over previous
"""Trainium2 Bass kernel for nn_AsymetricKernel (linear attention w/ InstanceNorm + 2D rotary).

Sharding: 8 cores = 4 batches x 2 head-groups (4 heads each). Fully independent
per core -- no collectives.

Per-core dataflow (PE compute in bf16, fp32 PSUM accumulation):
  - q is projected ONCE per head-pair in transposed layout [d, n]. Rotary's
    rotate-half is moved out of the data path entirely: u = (q*cos)^T D +
    (q*sin_sigma)^T D' where sin_sigma is a host-permuted sin table and
    D' = P^T D is the row-swapped dots matrix (one tiny PE matmul).
  - k/v are projected in token-partition layout [tok, k|v] with per-head
    (negated) mean columns from a parallel small matmul. Variance comes from
    one ACT square pass (PSUM->SBUF) + one grouped DVE reduce; the stats soup
    is fused with scalar_tensor_tensor ops, all unit-stride.
  - InstanceNorm scales combine as rc = 1/(sd_k*sd_v) applied to k only; v's
    mean rides as a -mv column in a stride-66 v layout so one matmul per
    (chunk, head-pair) accumulates both dots and the mean correction, which is
    folded back per-partition in the finalize.
  - k_rot = (k*rc + mcs)*cos + sigma-swap(...)*sin is pre-added so dots needs
    a single accumulating matmul per (chunk, head-pair).
  - Pass 2 streams 512-token chunks of t1/t2 against the stationary 128x128
    dots matrices: u^T accumulates in PSUM [e, tok], copied to bf16 and
    DMA'd out transposed; the host transposes back and upcasts.
"""

import numpy as np
import ml_dtypes

B, N, DIM, H, DH = 4, 8192, 512, 8, 64
HG = 2              # head groups (cores per batch) / head-pairs per core
HPG = H // HG       # heads per group = 4
E = HPG * DH        # 256 output cols per core
NT = 16             # n-tiles of 512
CPT = 4             # 128-chunks per n-tile
NCHUNK = NT * CPT   # 64
CC = DIM // 128     # 4 contraction chunks
VW = DH + 2         # v columns per head in dots rhs: 64 vals + (-mv) + pad

_cache = {}


def _build_program():
    import concourse.tile as tile
    from concourse import bacc, mybir
    from contextlib import ExitStack

    f32 = mybir.dt.float32
    bf16 = mybir.dt.bfloat16
    AX = mybir.AxisListType
    OP = mybir.AluOpType

    nc = bacc.Bacc(target_bir_lowering=False)
    uxT = nc.declare_dram_parameter("uxT", [NT, 128, CC * 512], bf16, isOutput=False)
    wq = nc.declare_dram_parameter("wq", [128, CC * E], bf16, isOutput=False)
    wkv = nc.declare_dram_parameter("wkv", [128, CC * 2 * E], bf16, isOutput=False)
    wm = nc.declare_dram_parameter("wm", [128, CC * 2 * HPG], bf16, isOutput=False)
    ctab = nc.declare_dram_parameter("ctab", [128, 2, N], bf16, isOutput=False)
    cptab = nc.declare_dram_parameter("cptab", [128, 2, NCHUNK * DH], bf16, isOutput=False)
    Pm = nc.declare_dram_parameter("Pm", [128, 128], bf16, isOutput=False)
    out = nc.declare_dram_parameter("out", [HG * 128, N], bf16, isOutput=True)

    with ExitStack() as ctx:
        tc = ctx.enter_context(tile.TileContext(nc))
        consts = ctx.enter_context(tc.tile_pool(name="consts", bufs=1))
        store = ctx.enter_context(tc.tile_pool(name="store", bufs=1))

        # ---- persistent SBUF ----
        wq_sb = consts.tile([128, CC, E], bf16)
        nc.sync.dma_start(wq_sb[:].rearrange("p c e -> p (c e)"), wq[:])
        wkv_sb = consts.tile([128, CC, 2 * E], bf16)
        nc.sync.dma_start(wkv_sb[:].rearrange("p c e -> p (c e)"), wkv[:])
        wm_sb = consts.tile([128, CC, 2 * HPG], bf16)
        nc.sync.dma_start(wm_sb[:].rearrange("p c e -> p (c e)"), wm[:])
        P_sb = consts.tile([128, 128], bf16)
        nc.sync.dma_start(P_sb[:], Pm[:])

        t1T_sb = store.tile([128, HG, N], bf16)   # (q * cos)^T per head-pair
        t2T_sb = store.tile([128, HG, N], bf16)   # (q * sin_sigma)^T
        D_sb = store.tile([128, HG, 128], bf16)   # block-diag dots per pair
        Dp_sb = store.tile([128, HG, 128], bf16)  # row-swapped dots
        gA = store.tile([128, HG], f32)           # -mv correction cols
        nc.vector.memset(D_sb[:], 0.0)

        with ExitStack() as p1:
            uxp = p1.enter_context(tc.tile_pool(name="uxp", bufs=3))
            ctp = p1.enter_context(tc.tile_pool(name="ctp", bufs=3))
            cpp = p1.enter_context(tc.tile_pool(name="cpp", bufs=3))
            sqp = p1.enter_context(tc.tile_pool(name="sqp", bufs=2))
            ksp = p1.enter_context(tc.tile_pool(name="ksp", bufs=2))
            kch = p1.enter_context(tc.tile_pool(name="kch", bufs=2))
            kdp = p1.enter_context(tc.tile_pool(name="kdp", bufs=5))
            stp = p1.enter_context(tc.tile_pool(name="stp", bufs=2))
            qps = p1.enter_context(tc.tile_pool(name="qps", bufs=2, space="PSUM"))
            kvps = p1.enter_context(tc.tile_pool(name="kvps", bufs=2, space="PSUM"))
            mps = p1.enter_context(tc.tile_pool(name="mps", bufs=1, space="PSUM"))
            dps = p1.enter_context(tc.tile_pool(name="dps", bufs=1, space="PSUM"))

            # dots accumulator: head-pair eb in partitions (g*64+d), columns
            # [g*66 + (v-dims 64, -mv col, pad)]; off-diagonal blocks are junk.
            dA = dps.tile([128, HG, 2 * VW], f32)

            pend = {}

            def emit_dots(t):
                krot_t, vD_t = pend.pop(t)
                for ci in range(CPT):
                    for eb in range(HG):
                        # start/stop once per BANK: start=True clears has_written
                        # bank-wide, so per-eb starts would wipe the other
                        # group's partial accumulation.
                        nc.tensor.matmul(
                            dA[:, eb, :], krot_t[:, ci, 2 * eb:2 * eb + 2, :],
                            vD_t[:, ci, 2 * eb:2 * eb + 2, :],
                            start=(t == 0 and ci == 0 and eb == 0),
                            stop=(t == NT - 1 and ci == CPT - 1 and eb == HG - 1))

            for nt in range(NT):
                ns = nt * 512
                ux_t = uxp.tile([128, CC, 512], bf16)
                nc.sync.dma_start(ux_t[:].rearrange("p c n -> p (c n)"), uxT[nt, :, :])
                ct_t = ctp.tile([128, 2, 512], bf16)
                nc.sync.dma_start(ct_t[:], ctab[:, :, ns:ns + 512])
                cp_t = cpp.tile([128, 2, CPT, DH], bf16)
                nc.sync.dma_start(cp_t[:].rearrange("p a c d -> p a (c d)"),
                                  cptab[:, :, nt * CPT * DH:(nt + 1) * CPT * DH])

                # ---- transposed q projection + rotary premuls ----
                # one ACT copy releases the PSUM bank fast; muls run bf16 2x
                for eb in range(HG):
                    qp = qps.tile([128, 512], f32, tag="qp")
                    for cc in range(CC):
                        nc.tensor.matmul(
                            qp[:], wq_sb[:, cc, eb * 128:(eb + 1) * 128],
                            ux_t[:, cc, :],
                            start=(cc == 0), stop=(cc == CC - 1))
                    qsb = stp.tile([128, 512], bf16, tag="qsb")
                    nc.scalar.copy(qsb[:], qp[:])
                    nc.vector.tensor_mul(
                        t1T_sb[:, eb, ns:ns + 512], qsb[:], ct_t[:, 0, :])
                    nc.vector.tensor_mul(
                        t2T_sb[:, eb, ns:ns + 512], qsb[:], ct_t[:, 1, :])

                # ---- k/v + mean projections, PSUM->SBUF per half-tile ----
                mp = mps.tile([128, CPT, 2 * HPG], f32, tag="mp")
                sq = sqp.tile([128, CPT, 2 * HPG, DH], bf16, tag="sq")
                ksb = ksp.tile([128, CPT, HPG, DH], bf16, tag="ksb")
                vD = kdp.tile([128, CPT, HPG, VW], bf16, tag="vD")
                for hf in range(2):
                    kvp = kvps.tile([128, 2, 2 * E], f32, tag="kvp")
                    for ci2 in range(2):
                        ci = hf * 2 + ci2
                        for cc in range(CC):
                            nc.tensor.matmul(
                                kvp[:, ci2, :], ux_t[:, cc, ci * 128:(ci + 1) * 128],
                                wkv_sb[:, cc, :],
                                start=(cc == 0), stop=(cc == CC - 1))
                            nc.tensor.matmul(
                                mp[:, ci, :], ux_t[:, cc, ci * 128:(ci + 1) * 128],
                                wm_sb[:, cc, :],
                                start=(cc == 0), stop=(cc == CC - 1))
                    # copies first (release the PSUM bank fast), squares from
                    # the SBUF copies afterwards (ACT for k, GPS for v)
                    hsl = slice(2 * hf, 2 * hf + 2)
                    nc.scalar.copy(ksb[:, hsl],
                                   kvp[:, :, 0:E].rearrange("p c (g d) -> p c g d", g=HPG))
                    nc.scalar.copy(vD[:, hsl, :, 0:DH],
                                   kvp[:, :, E:2 * E].rearrange("p c (g d) -> p c g d", g=HPG))
                    nc.scalar.square(sq[:, hsl, 0:HPG],
                                     ksb[:, hsl])
                    nc.gpsimd.tensor_mul(sq[:, hsl, HPG:2 * HPG],
                                         vD[:, hsl, :, 0:DH], vD[:, hsl, :, 0:DH])

                # ---- whole-tile stats, all unit-stride ----
                mpc = stp.tile([128, CPT, 2 * HPG], f32, tag="mpc")
                nc.scalar.copy(mpc[:], mp[:])
                ss = stp.tile([128, CPT, 2 * HPG], f32, tag="ss")
                nc.vector.tensor_reduce(out=ss[:], in_=sq[:], axis=AX.X, op=OP.add)
                negmu2 = stp.tile([128, CPT, 2 * HPG], f32, tag="nmu")
                nc.vector.scalar_tensor_tensor(
                    negmu2[:], mpc[:], -1.0, mpc[:], OP.mult, OP.mult)
                var = stp.tile([128, CPT, 2 * HPG], f32, tag="var")
                nc.vector.scalar_tensor_tensor(
                    var[:], ss[:], 1.0 / DH, negmu2[:], OP.mult, OP.add)
                sd = stp.tile([128, CPT, 2 * HPG], f32, tag="sd")
                nc.scalar.sqrt(sd[:], var[:])
                sdp = stp.tile([128, CPT, HPG], f32, tag="sdp")
                nc.vector.tensor_mul(sdp[:], sd[:, :, 0:HPG], sd[:, :, HPG:2 * HPG])
                rc = stp.tile([128, CPT, HPG], f32, tag="rc")
                nc.vector.reciprocal(rc[:], sdp[:])
                mcs = stp.tile([128, CPT, HPG], f32, tag="mcs")
                nc.gpsimd.tensor_mul(mcs[:], mpc[:, :, 0:HPG], rc[:])
                # v mean col (+ pad col, same value, junk output never read)
                nc.scalar.copy(vD[:, :, :, DH:DH + 2],
                               mpc[:, :, HPG:2 * HPG].unsqueeze(-1).broadcast_to(
                                   [128, CPT, HPG, 2]))

                # ---- k chain: normalize + rotary, pre-added ----
                ktn = kch.tile([128, CPT, HPG, DH], bf16, tag="ktn")
                nc.gpsimd.tensor_mul(
                    ktn[:], ksb[:],
                    rc[:].unsqueeze(-1).broadcast_to([128, CPT, HPG, DH]))
                nc.gpsimd.tensor_add(
                    ktn[:], ktn[:],
                    mcs[:].unsqueeze(-1).broadcast_to([128, CPT, HPG, DH]))
                k1 = kch.tile([128, CPT, HPG, DH], bf16, tag="k1")
                nc.vector.tensor_mul(
                    k1[:], ktn[:],
                    cp_t[:, 0].unsqueeze(2).broadcast_to([128, CPT, HPG, DH]))
                k2 = kch.tile([128, CPT, HPG, DH], bf16, tag="k2")
                ktn_r = ktn[:].rearrange("p c g (b h s) -> p c g b h s", b=2, h=2, s=16)
                k2_r = k2[:].rearrange("p c g (b h s) -> p c g b h s", b=2, h=2, s=16)
                spb = cp_t[:, 1].rearrange("p c (b h s) -> p c b h s", b=2, h=2, s=16
                                           ).unsqueeze(2).broadcast_to(
                                               [128, CPT, HPG, 2, 2, 16])
                # ISA TensorTensor allows max 3 free dims -> one op per (b, h)
                for bb in range(2):
                    for hh in range(2):
                        nc.vector.tensor_mul(
                            k2_r[:, :, :, bb, hh, :],
                            ktn_r[:, :, :, bb, 1 - hh, :],
                            spb[:, :, :, bb, hh, :])
                krot = kdp.tile([128, CPT, HPG, DH], bf16, tag="krot")
                nc.vector.tensor_add(krot[:], k1[:], k2[:])
                pend[nt] = (krot, vD)
                # 3-tile lag: the elementwise chain has ~11us latency across
                # engine FIFOs; this slack keeps the PE from ever stalling on
                # it (which would re-throttle the HAM clock gate)
                if nt > 2:
                    emit_dots(nt - 3)
            emit_dots(NT - 3)
            emit_dots(NT - 2)
            emit_dots(NT - 1)

            # ---- finalize dots: fold -mv col, scale 1/N, cast bf16 ----
            for eb in range(HG):
                for g in range(2):
                    psl = slice(g * 64, (g + 1) * 64)
                    col = g * VW + DH
                    nc.scalar.copy(gA[psl, eb:eb + 1], dA[psl, eb, col:col + 1])
            for eb in range(HG):
                for g in range(2):
                    psl = slice(g * 64, (g + 1) * 64)
                    nc.vector.tensor_scalar(
                        out=D_sb[psl, eb, g * 64:(g + 1) * 64],
                        in0=dA[psl, eb, g * VW:g * VW + DH],
                        scalar1=gA[psl, eb:eb + 1], scalar2=1.0 / N,
                        op0=OP.add, op1=OP.mult)

        # ---- pass 2: D' = P^T D, then u^T = D^T t1 + D'^T t2 ----
        with ExitStack() as p2:
            dpps = p2.enter_context(tc.tile_pool(name="dpps", bufs=1, space="PSUM"))
            ups = p2.enter_context(tc.tile_pool(name="ups", bufs=3, space="PSUM"))
            uout = p2.enter_context(tc.tile_pool(name="uout", bufs=4))
            dps2 = dpps.tile([128, HG, 128], f32)
            for eb in range(HG):
                nc.tensor.matmul(dps2[:, eb, :], P_sb[:], D_sb[:, eb, :],
                                 start=True, stop=True)
                nc.vector.tensor_copy(Dp_sb[:, eb, :], dps2[:, eb, :])
            for eb in range(HG):
                for cs in range(NT):
                    nsl = slice(cs * 512, (cs + 1) * 512)
                    up = ups.tile([128, 512], f32, tag="up")
                    nc.tensor.matmul(up[:], D_sb[:, eb, :], t1T_sb[:, eb, nsl],
                                     start=True, stop=False)
                    nc.tensor.matmul(up[:], Dp_sb[:, eb, :], t2T_sb[:, eb, nsl],
                                     start=False, stop=True)
                    u_sb = uout.tile([128, 512], bf16)
                    if cs % 2 == 0:
                        nc.vector.tensor_copy(u_sb[:], up[:])
                    else:
                        nc.scalar.copy(u_sb[:], up[:])
                    nc.sync.dma_start(out[eb * 128:(eb + 1) * 128, nsl], u_sb[:])

    nc.finalize()
    return nc


def _host_prep(u_x, pos_x, Wq, Wk, Wv):
    bf = ml_dtypes.bfloat16
    invf = 1.0 / 10000.0 ** (np.arange(0, 32, 2, dtype=np.float64) / 32)
    t64 = pos_x[0].astype(np.float64) * 64.0
    fx = t64[:, 0:1] * invf[None, :]
    fy = t64[:, 1:2] * invf[None, :]
    cx, sx = np.cos(fx), np.sin(fx)
    cy, sy = np.cos(fy), np.sin(fy)
    cosf = np.concatenate([cx, cx, cy, cy], 1).astype(np.float32)     # c[n, 64]
    stil = np.concatenate([-sx, sx, -sy, sy], 1).astype(np.float32)   # rho*sin
    sig = np.arange(64) ^ 16

    def chunked(t):  # [N, 64] -> [128, NCHUNK*64], partition = token % 128
        return np.ascontiguousarray(
            t.reshape(NCHUNK, 128, DH).transpose(1, 0, 2).reshape(128, -1))

    cptab = np.stack([chunked(cosf), chunked(stil)], axis=1).astype(bf)
    ctab = np.stack([np.tile(cosf.T, (2, 1)),
                     np.tile(stil[:, sig].T, (2, 1))], axis=1).astype(bf)
    Pmh = np.eye(128, dtype=np.float32)[np.arange(128) ^ 16].astype(bf)

    def wlayout(wT):  # [512, E'] -> [128, CC*E'] partition-native
        Ep = wT.shape[1]
        return np.ascontiguousarray(
            wT.reshape(CC, 128, Ep).transpose(1, 0, 2).reshape(128, -1)).astype(bf)

    in_maps = []
    for b in range(B):
        uxtb = np.ascontiguousarray(
            u_x[b].reshape(NT, 512, CC, 128).transpose(0, 3, 2, 1).reshape(
                NT, 128, CC * 512)).astype(bf)
        for hg in range(HG):
            sl = slice(hg * E, (hg + 1) * E)
            wbar_k = Wk[sl].reshape(HPG, DH, DIM).mean(1).T            # [512, 4]
            wbar_v = Wv[sl].reshape(HPG, DH, DIM).mean(1).T
            in_maps.append({
                "uxT": uxtb,
                "wq": wlayout(Wq[sl].T.astype(np.float32)),
                "wkv": wlayout(np.concatenate([Wk[sl].T, Wv[sl].T], 1)),
                "wm": wlayout(np.concatenate([-wbar_k, -wbar_v], 1)),
                "ctab": ctab, "cptab": cptab, "Pm": Pmh,
            })
    return in_maps


def kernel(u_x, pos_x, Wq, Wk, Wv, _trace=False, _trace_dir=None):
    from concourse.bass_utils import run_bass_kernel_spmd

    if "nc" not in _cache:
        _cache["nc"] = _build_program()
    nc = _cache["nc"]

    in_maps = _host_prep(
        np.asarray(u_x, np.float32), np.asarray(pos_x, np.float32),
        np.asarray(Wq, np.float32), np.asarray(Wk, np.float32),
        np.asarray(Wv, np.float32))

    kw = {}
    if _trace:
        kw = {"trace": True, "tmpdir": _trace_dir}
    res = run_bass_kernel_spmd(nc, in_maps, core_ids=list(range(8)), **kw)
    _cache["last_result"] = res

    out = np.empty((B, N, H * DH), np.float32)
    for i in range(8):
        b, hg = divmod(i, HG)
        out[b, :, hg * E:(hg + 1) * E] = np.asarray(
            res.results[i]["out"], np.float32).T
    return out
